# revision 48
# baseline (speedup 1.0000x reference)
"""Multihead attention (B=2, S=2048, E=1024, H=16) on 8 TRN2 cores.

Sharding: tensor-parallel over heads — core c computes heads {2c, 2c+1}
(dout = 128 columns of the QKV projections) for the full sequence, then its
partial contribution to the output projection; the host sums the 8 partials
and adds the output bias.

Device layout (per core):
  activations are pre-transposed on host to x^T [E, B*S] (bf16, packed
  [128, NST, KE, SEQT] so each seq-tile streams as ONE 8KB-line DMA) so the
  projection matmuls contract E on the partition dim.  QKV projections
  produce Q^T/K^T/V^T [128, 4096] in SBUF (bf16).  Attention per
  (batch, head-pair) computes scores^T [kpos, q] per head (lhsT = K^T
  slice, rhs = Q^T slice), exponentiates on the scalar engine (fp32 psum
  in, bf16 out), and multiplies by V via matmul with lhsT = [V | ones] so
  the softmax denominator falls out of the same accumulation (row 64 of
  the PSUM result).  context^T is normalized with a batched reciprocal +
  a PE-replicated row, cast to bf16, and the output projection runs fully
  in bf16 (fp32 psum), writing fp16 partials to HBM.

Scheduling: the attention phase is scalar-engine(exp)-bound, so all other
work hides inside it — projection sub-steps (batch overlap) and output-
projection blocks are emitted into fixed slots of the attention t-loop;
each step's normalization is split into psum-freeing copies (t=0 of the
next step), the batched reciprocal (t=4), and the replicate+multiply
(t=12) so the in-order PE queue never waits on the DVE.  V^T transposes
go through the DMA XBAR plus a Pool-engine scatter (GPSIMD cannot read
PSUM, so the output-projection drains stay on DVE/ACT).
"""

import numpy as np
import ml_dtypes

# Problem constants (hardcoded per the task contract).
B, S, E, H = 2, 2048, 1024, 16
D = E // H          # 64
NSEQ = B * S        # 4096
NCORES = 8
DOUT = E // NCORES  # 128 = 2 heads x 64
KE = E // 128       # 8 contraction tiles over E
SEQT = 512          # seq tile for projections / q-block for attention
NST = NSEQ // SEQT  # 8
QB = S // SEQT      # 4 q-blocks per batch
KT = S // 128       # 16 kpos tiles per batch
ISD = float(D) ** -0.5

_PROGRAM = None


# ---------------------------------------------------------------------------
# Workarounds for this walrus build: at most ONE sync wait per instruction is
# reliably accepted ("Too many sync wait commands").  (1) tile's final drain
# gets one wait per logical proc — split them over single-wait SP NOPs;
# (2) a general post-pass moves any instruction's excess waits onto
# preceding same-engine NOPs (engine program order preserves semantics).
# ---------------------------------------------------------------------------


def _install_tile_drain_patch():
    import concourse.mybir as mybir
    import concourse.tile as tile
    from concourse.tile import ScopedClock

    if getattr(tile.TileContext, "_drain_patch_installed", False):
        return

    def _patched_drain_and_barrier(self, tick_clock, wait_clock):
        nc = self.nc
        carrier = nc.sync.nop(nofuse=True)
        wait_clock.add_sem_waits(
            carrier.ins, ScopedClock({None: tick_clock.global_clock})
        )
        si = carrier.ins.sync_info
        waits = list(si.on_wait) if si and si.on_wait else []
        ups = list(si.on_update) if si and si.on_update else []
        if len(waits) > 1:
            carrier.ins.sync_info = mybir.SyncInfo(on_wait=[waits[0]], on_update=ups)
            for w in waits[1:]:
                n2 = nc.sync.nop(nofuse=True)
                n2.ins.sync_info = mybir.SyncInfo(on_wait=[w], on_update=[])
        nc.sync.drain()
        nc.all_engine_barrier()
        popped = nc._tile_sem_poison_stack.pop()
        assert popped is self._sem_poison
        nc.clear_and_free_semaphores(list(self.sems.allocated().values()))
        nc.all_engine_barrier()

    tile.TileContext._drain_and_barrier = _patched_drain_and_barrier
    tile.TileContext._drain_patch_installed = True


MAX_WAITS = 1


def _split_excess_waits(nc):
    import concourse.mybir as mybir

    for bb in nc.main_func.blocks:
        il = list(bb.instructions)
        out = []
        changed = False
        for ins in il:
            si = ins.sync_info
            waits = list(si.on_wait) if si and si.on_wait else []
            if len(waits) > MAX_WAITS:
                changed = True
                extras = waits[: len(waits) - MAX_WAITS]
                keep = waits[len(extras):]
                for i in range(0, len(extras), MAX_WAITS):
                    chunk = extras[i : i + MAX_WAITS]
                    nop = mybir.InstNoOp(
                        name=nc.get_next_instruction_name(), ins=[], outs=[]
                    )
                    nop.engine = ins.engine
                    nop.sync_info = mybir.SyncInfo(on_wait=chunk, on_update=[])
                    out.append(nop)
                ins.sync_info = mybir.SyncInfo(
                    on_wait=keep, on_update=list(si.on_update) if si.on_update else []
                )
            out.append(ins)
        if changed:
            bb.instructions = out


def _build_program():
    import concourse.bass as bass
    import concourse.mybir as mybir
    import concourse.tile as tile
    from concourse.masks import make_identity

    _install_tile_drain_patch()

    f32 = mybir.dt.float32
    f32r = mybir.dt.float32r
    bf16 = mybir.dt.bfloat16
    fp16 = mybir.dt.float16

    nc = bass.Bass("TRN2", target_bir_lowering=False, debug=False)

    # DRAM I/O (per core).  x packed [128, NST, KE, SEQT] so one seq-tile is
    # a single DMA with 8KB contiguous per partition; weights packed
    # [128, KE, DOUT] (2KB lines).
    xq = nc.dram_tensor("xq", [128, NST, KE, SEQT], bf16, kind="ExternalInput").ap()
    xk = nc.dram_tensor("xk", [128, NST, KE, SEQT], bf16, kind="ExternalInput").ap()
    xv = nc.dram_tensor("xv", [128, NST, KE, SEQT], bf16, kind="ExternalInput").ap()
    wq = nc.dram_tensor("wq", [128, KE, DOUT], bf16, kind="ExternalInput").ap()
    wk = nc.dram_tensor("wk", [128, KE, DOUT], bf16, kind="ExternalInput").ap()
    wv = nc.dram_tensor("wv", [128, KE, DOUT], bf16, kind="ExternalInput").ap()
    wo = nc.dram_tensor("wo", [DOUT, E], bf16, kind="ExternalInput").ap()
    bq = nc.dram_tensor("bq", [DOUT, 1], f32, kind="ExternalInput").ap()
    bk = nc.dram_tensor("bk", [DOUT, 1], f32, kind="ExternalInput").ap()
    bv = nc.dram_tensor("bv", [DOUT, 1], f32, kind="ExternalInput").ap()
    out = nc.dram_tensor("out", [NSEQ, E], fp16, kind="ExternalOutput").ap()

    with tile.TileContext(nc) as tc:
        with (
            nc.allow_low_precision(reason="bf16 attention pipeline"),
            tc.tile_pool(name="consts", bufs=1) as consts,
            tc.tile_pool(name="persist", bufs=1) as persist,
            tc.tile_pool(name="xstream", bufs=10) as xstream,
            tc.tile_pool(name="ptp", bufs=6) as ptp,
            tc.tile_pool(name="vtp", bufs=3) as vtp,
            tc.tile_pool(name="outp", bufs=4) as outp,
            tc.tile_pool(name="small", bufs=8) as small,
            tc.tile_pool(name="pp_ps", bufs=2, space="PSUM") as pp_ps,
            tc.tile_pool(name="sc_ps", bufs=2, space="PSUM") as sc_ps,
            tc.tile_pool(name="cx_ps", bufs=2, space="PSUM") as cx_ps,
        ):
            # ---- weights / first x tiles; split dispatch over the two
            # hardware DGE queues (SP + ACT) so startup loads parallelize ----
            w_sb = {}
            b_sb = {}
            xdram = {"q": xq, "k": xk, "v": xv}
            xtiles = {}
            # steady-state x prefetches ride SP only: a dispatch on the
            # scalar DGE queue costs the exp-bound scalar engine ~0.7us each
            # and can head-of-line-block exps; the scalar queue is used only
            # during the exp-free prefix
            xload_eng = [nc.sync, nc.sync]

            def load_w(name, wdram, bdram, eng):
                wt = persist.tile([128, KE, DOUT], bf16, tag=f"w{name}", name=f"w{name}")
                eng.dma_start(wt[:], wdram[:])
                w_sb[name] = wt
                bt = persist.tile([DOUT, 1], f32, tag=f"b{name}", name=f"b{name}")
                eng.dma_start(bt[:], bdram[:])
                b_sb[name] = bt

            def load_x(name, st, eng, split=False):
                xt = xstream.tile([128, KE, SEQT], bf16, tag="xs", name="xt")
                if split:
                    # halves so the first projection's k-loop starts sooner
                    eng.dma_start(xt[:, 0:4, :], xdram[name][:, st, 0:4, :])
                    eng.dma_start(xt[:, 4:8, :], xdram[name][:, st, 4:8, :])
                else:
                    eng.dma_start(xt[:], xdram[name][:, st, :, :])
                xtiles[(name, st)] = xt

            load_w("q", wq, bq, nc.sync)
            load_x("q", 0, nc.sync, split=True)
            load_w("k", wk, bk, nc.scalar)
            load_x("k", 0, nc.scalar, split=True)
            load_x("v", 0, nc.sync, split=True)
            load_w("v", wv, bv, nc.scalar)
            load_x("q", 1, nc.scalar)
            load_x("k", 1, nc.sync)
            load_x("v", 1, nc.scalar)

            # ---- constants / persistent SBUF state ----
            ident_f32 = consts.tile([128, 128], f32)
            make_identity(nc, ident_f32[:])
            ident = consts.tile([128, 128], bf16)
            nc.vector.tensor_copy(ident[:], ident_f32[:])
            onesf = consts.tile([128, 1], f32)
            nc.vector.memset(onesf[:], 1.0)
            # denominator-replication expander: out[p] = rec[64 * (p // 64)]
            # (partition bases must be 0/32/64, so the two source rows sit at
            # partitions 0 and 64)
            expand_f = consts.tile([D + 1, 128], f32)
            nc.vector.memset(expand_f[:], 0.0)
            nc.vector.memset(expand_f[0:1, 0:D], 1.0)
            nc.vector.memset(expand_f[D : D + 1, D:128], 1.0)
            expand = consts.tile([D + 1, 128], f32r)
            nc.vector.tensor_copy(expand[:], expand_f[:])

            wo_sb = persist.tile([DOUT, E], bf16, tag="wo")
            nc.scalar.dma_start(wo_sb[:], wo[:])

            qt_sb = persist.tile([128, NSEQ], bf16, tag="qt")
            kt_sb = persist.tile([128, NSEQ], bf16, tag="kt")
            vt_sb = persist.tile([128, NSEQ], bf16, tag="vt")
            xT_sb = {"q": qt_sb, "k": kt_sb, "v": vt_sb}
            # [V | ones] per (kpos chunk, head): [128, 32, 2, 65] bf16
            v_sb = persist.tile([128, NSEQ // 128, 2, D + 1], bf16, tag="vn")
            nc.vector.tensor_copy(
                v_sb[:, :, :, D], onesf[:, 0:1].broadcast_to([128, NSEQ // 128, 2])
            )
            ctxT_sb = persist.tile([128, NSEQ], bf16, tag="ctxT")

            def proj_items(st):
                # split one seq-tile's projections into schedulable items
                def proj_name(name):
                    def run():
                        if (name, st) not in xtiles:
                            load_x(name, st, xload_eng[st % 2])
                        for tgt in (st + 1, st + 2):
                            if tgt < NST and ("q", tgt) not in xtiles:
                                for nm in ("q", "k", "v"):
                                    load_x(nm, tgt, xload_eng[tgt % 2])
                        sl = bass.ts(st, SEQT)
                        xt = xtiles.pop((name, st))
                        ps = pp_ps.tile([128, SEQT], f32, tag="pp", name=f"pp{st}{name}")
                        for k in range(KE):
                            nc.tensor.matmul(
                                ps[:],
                                lhsT=w_sb[name][:, k, :],
                                rhs=xt[:, k, :],
                                start=(k == 0),
                                stop=(k == KE - 1),
                            )
                        nc.vector.tensor_scalar_add(
                            xT_sb[name][:, sl], ps[:], b_sb[name][:, 0:1]
                        )
                        if name == "v":
                            # batch-0 chunks are needed within ~1 step, so
                            # they transpose on the PE (low latency); batch-1
                            # chunks have whole-phase slack and ride the DMA
                            # XBAR + Pool scatter instead (strided XBAR dsts
                            # are broken, hence the contiguous temp)
                            for ci in range(
                                st * (SEQT // 128), (st + 1) * (SEQT // 128)
                            ):
                                if st < 4:
                                    tp = pp_ps.tile(
                                        [128, 128], bf16, tag="pp", name="tp"
                                    )
                                    nc.tensor.transpose(
                                        tp[:], vt_sb[:, bass.ts(ci, 128)], ident[:]
                                    )
                                    for h in range(2):
                                        nc.vector.tensor_copy(
                                            v_sb[:, ci, h, 0:D],
                                            tp[:, bass.ts(h, D)],
                                        )
                                else:
                                    vtmp = vtp.tile(
                                        [128, 128], bf16, tag="vtmp", name="vtmp"
                                    )
                                    nc.sync.dma_start_transpose(
                                        vtmp[:], vt_sb[:, bass.ts(ci, 128)]
                                    )
                                    for h in range(2):
                                        nc.gpsimd.tensor_copy(
                                            v_sb[:, ci, h, 0:D],
                                            vtmp[:, bass.ts(h, D)],
                                        )

                    return run

                return [proj_name(n) for n in ("q", "k", "v")]

            WORK_SLOTS = (2, 4, 6, 8, 12, 14)
            FIRST_SLOTS = (1, 2, 3, 4, 5, 6, 8, 10, 12)

            def attn_step(b, qb, fins_prev, work, slots=WORK_SLOTS):
                qsl = bass.ds(b * S + qb * SEQT, SEQT)
                ctx = [None, None]
                for h in range(2):
                    ctx[h] = cx_ps.tile([D + 1, SEQT], f32, tag="cx", name=f"ctx{h}")
                fin_at = {0: 0, 4: 1, 12: 2}

                def make_pv(t, pt):
                    # PV(t) is emitted during iteration t+1 (software
                    # pipelining): the PE never queues behind exp(t), which
                    # is still running while scores(t+1) execute
                    def run():
                        for h in range(2):
                            nc.tensor.matmul(
                                ctx[h][:],
                                lhsT=v_sb[:, b * KT + t, h, :],
                                rhs=pt[:, bass.ts(h, SEQT)],
                                start=(t == 0),
                                stop=(t == KT - 1),
                            )

                    return run

                pv_prev = None
                for t in range(KT):
                    ksl = bass.ds(b * S + t * 128, 128)
                    # both heads' scores into one 2-bank psum tile; a single
                    # merged exp halves the scalar engine's per-op overhead
                    sc = sc_ps.tile([128, 2 * SEQT], f32, tag="sc", name="sc")
                    for h in range(2):
                        hsl = bass.ts(h, D)
                        nc.tensor.matmul(
                            sc[:, bass.ts(h, SEQT)],
                            lhsT=kt_sb[hsl, ksl],
                            rhs=qt_sb[hsl, qsl],
                            start=True,
                            stop=True,
                        )
                    if fins_prev is not None and t in fin_at:
                        fins_prev[fin_at[t]]()
                    pt = ptp.tile([128, 2 * SEQT], bf16, tag="pt", name="pt")
                    nc.scalar.activation(
                        pt[:], sc[:], mybir.ActivationFunctionType.Exp, scale=ISD
                    )
                    if pv_prev is not None:
                        pv_prev()
                    if t in slots and work:
                        work.pop(0)()
                    pv_prev = make_pv(t, pt)
                pv_prev()

                ctmp = [None, None]
                recbox = [None]

                def fin_fast():
                    # one copy per head frees the ctx psum banks quickly so
                    # the next step's PV matmuls aren't blocked
                    for h in range(2):
                        ctmp[h] = small.tile([D + 1, SEQT], f32, tag="ctmp", name="ctmp")
                        nc.vector.tensor_copy(ctmp[h][:], ctx[h][:])

                def fin_mid():
                    # batched reciprocal of both heads' denominators, parked
                    # at partitions 0 and 64 (legal AP bases); rows 1..63 are
                    # memset to 1.0 so the full-tile reciprocal reads no
                    # garbage and the expander contracts them against zeros
                    den = small.tile([D + 1, SEQT], f32, tag="den", name="den")
                    nc.vector.memset(den[:], 1.0)
                    nc.vector.tensor_copy(den[0:1, :], ctmp[0][D : D + 1, :])
                    nc.vector.tensor_copy(den[D : D + 1, :], ctmp[1][D : D + 1, :])
                    rec = small.tile([D + 1, SEQT], f32r, tag="rec", name="rec")
                    nc.vector.reciprocal(rec[:], den[:])
                    recbox[0] = rec

                def fin_slow():
                    # replicate both heads' 1/denom rows in one matmul
                    rrep = pp_ps.tile([128, SEQT], f32, tag="pp", name="rrep")
                    nc.tensor.matmul(
                        rrep[:], lhsT=expand[:], rhs=recbox[0][:],
                        start=True, stop=True,
                    )
                    for h in range(2):
                        hsl = bass.ts(h, D)
                        nc.vector.tensor_tensor(
                            out=ctxT_sb[hsl, qsl],
                            in0=ctmp[h][0:D, :],
                            in1=rrep[hsl, :],
                            op=mybir.AluOpType.mult,
                        )

                return fin_fast, fin_mid, fin_slow

            def outproj_item(m, engs=(nc.vector, nc.vector)):
                # GPSIMD can't read PSUM, so drains go to DVE during the
                # (scalar-bound) attention phase, DVE+ACT in the tail
                def run():
                    ob = outp.tile([128, E], fp16, tag="ob", name="ob")
                    for n, eng in enumerate(engs):
                        ps = pp_ps.tile([128, SEQT], f32, tag="pp", name="ops")
                        nc.tensor.matmul(
                            ps[:],
                            lhsT=ctxT_sb[:, bass.ts(m, 128)],
                            rhs=wo_sb[:, bass.ts(n, SEQT)],
                            start=True,
                            stop=True,
                        )
                        if eng is nc.scalar:
                            nc.scalar.activation(
                                ob[:, bass.ts(n, SEQT)], ps[:],
                                mybir.ActivationFunctionType.Copy,
                            )
                        else:
                            eng.tensor_copy(ob[:, bass.ts(n, SEQT)], ps[:])
                    nc.sync.dma_start(out[bass.ts(m, 128), :], ob[:])

                return run

            # ---- emission: streaming prefix (attention starts after only
            # the first seq-tile's projections; the rest of batch 0 streams
            # through the first attention step's work slots) ----
            for item in proj_items(0):
                item()
            fins = None
            work = []
            p1, p2, p3 = (proj_items(st) for st in (1, 2, 3))
            # k/v first (earliest attention deps); q-tiles are only needed
            # from the next step on
            work.extend([p1[1], p1[2], p2[1], p2[2], p1[0], p3[1], p3[2]])
            fins = attn_step(0, 0, fins, work, slots=FIRST_SLOTS)
            # q2/q3 are only read by steps (0,2)/(0,3); hosting them in
            # (0,1) unloads the overfull first step
            work.extend([p2[0], p3[0]])
            for qb in range(1, QB):       # b0 attention // b1 projections
                work.extend(proj_items(3 + qb))
                if qb >= 2:               # b0 out-proj becomes emit-legal
                    work.extend(outproj_item(m) for m in range(4 * (qb - 2), 4 * (qb - 1)))
                fins = attn_step(0, qb, fins, work)
            # b1 attention // remaining out-proj blocks.  m20..23 ride the
            # late slots (>= t12) of steps (1,2)/(1,3): their normalization
            # is emitted at t=10 of the immediately-preceding step, so FIFO
            # position >= 5 keeps emission legal.
            work.extend(proj_items(7))
            extra = {2: [20, 21], 3: [22, 23]}
            for qb in range(QB):
                lo = 4 + 4 * qb
                work.extend(outproj_item(m) for m in range(lo, lo + 4))
                work.extend(outproj_item(m) for m in extra.get(qb, []))
                fins = attn_step(1, qb, fins, work)
            # drain: final normalization, then the tail blocks (split psum
            # drains across DVE + ACT since exp work is done)
            for item in work:
                item()
            fins[0]()
            fins[1]()
            for m in range(24, 28):
                outproj_item(m, engs=(nc.vector, nc.scalar))()
            fins[2]()
            for m in range(28, 32):
                outproj_item(m, engs=(nc.vector, nc.scalar))()

    return nc


def _get_program():
    global _PROGRAM
    if _PROGRAM is None:
        _PROGRAM = _build_program()
    return _PROGRAM


def kernel(query, key, value, Wq, bq, Wk, bk, Wv, bv, Wo, bo):
    from concourse.bass_utils import run_bass_kernel_spmd

    nc = _get_program()
    if not getattr(nc, "_waits_split", False):
        _split_excess_waits(nc)
        nc._waits_split = True

    bf = ml_dtypes.bfloat16
    q2 = np.asarray(query, np.float32).reshape(NSEQ, E)
    k2 = np.asarray(key, np.float32).reshape(NSEQ, E)
    v2 = np.asarray(value, np.float32).reshape(NSEQ, E)

    # x^T [E, NSEQ] -> [128, NST, KE, SEQT] (partition-major, seq-tile
    # contiguous), rounded to bf16 on host (the bf16 matmul rounds anyway)
    def pack_x(x2):
        xT = np.ascontiguousarray(x2.T).astype(bf)
        return np.ascontiguousarray(
            xT.reshape(KE, 128, NST, SEQT).transpose(1, 2, 0, 3)
        )

    xqh = pack_x(q2)
    xkh = pack_x(k2)
    xvh = pack_x(v2)

    Wq = np.asarray(Wq, np.float32)
    Wk = np.asarray(Wk, np.float32)
    Wv = np.asarray(Wv, np.float32)
    Wo = np.asarray(Wo, np.float32)

    def pack_w(W, rsl):
        # lhsT for the projections: (W_c)^T [E, DOUT] -> [128, KE, DOUT]
        wT = np.ascontiguousarray(W[rsl, :].T).astype(bf)
        return np.ascontiguousarray(wT.reshape(KE, 128, DOUT).transpose(1, 0, 2))

    in_maps = []
    for c in range(NCORES):
        rsl = slice(DOUT * c, DOUT * (c + 1))
        in_maps.append(
            {
                "xq": xqh, "xk": xkh, "xv": xvh,
                "wq": pack_w(Wq, rsl),
                "wk": pack_w(Wk, rsl),
                "wv": pack_w(Wv, rsl),
                # rhs for the out-proj: rows c-range of Wo^T  [DOUT, E]
                "wo": np.ascontiguousarray(Wo[:, rsl].T).astype(bf),
                "bq": np.ascontiguousarray(np.asarray(bq, np.float32)[rsl]).reshape(DOUT, 1),
                "bk": np.ascontiguousarray(np.asarray(bk, np.float32)[rsl]).reshape(DOUT, 1),
                "bv": np.ascontiguousarray(np.asarray(bv, np.float32)[rsl]).reshape(DOUT, 1),
            }
        )

    res = run_bass_kernel_spmd(nc, in_maps, list(range(NCORES)), trace=False)
    acc = np.zeros((NSEQ, E), np.float32)
    for c in range(NCORES):
        acc += res.results[c]["out"].astype(np.float32)
    acc += np.asarray(bo, np.float32)[None, :]
    return acc.reshape(B, S, E)
